# revision 9
# baseline (speedup 1.0000x reference)
"""Sliding context-window gather kernel for Trainium2 (Bass/Tile).

Computes, for x[B=32, T=2000, C=80] and lengths[B]:
    out[b, t, c*11 + i] = x[b, t + i - 5, c]          (zero outside [0, T))
                          * (t < round(T * lengths[b]))
i.e. an 11-tap sliding-window gather along T with channel-major
interleave, masked by per-sample length.

Sharding: pure data-parallel, 4 samples per core across 8 cores, with a
host-side length-balanced permutation (greedy LPT on per-sample kept
rows) so every core stores a near-equal number of bytes.

Layout: host zero-pads x by 5 rows on each side of T; each sample is
loaded into SBUF as an overlapping-window view [125p x 26r x 80c] (one
DMA; partition p holds padded rows 16p..16p+25 => t = 16p-5..16p+20).

Compute: one fused op per folded q-row builds the full 880-wide
interleaved+masked output row contiguously:
    O[p, (c,i)] = X[p, q+i, c] * mask[p, q]
via tensor_scalar multiply with a per-partition scalar (the mask value)
and a transposed source access pattern (c outer stride 1, i inner
stride 80).  Rows are split between the vector and scalar(ACT) engines
(~1 us/row each, measured); aggregate compute is ~4x faster than the
store stream, so it fully hides.

DMA: this environment's HWDGE queues (sync/scalar) share only 5 SDMA
engines (~120 GB/s combined), while the gpsimd SWDGE queue spreads
across all 16 engines (~190 GB/s measured).  Loads are prefetched up
front on the sync HWDGE ring; stores go through SWDGE.  Stores are
issued per 25-partition block (400 t-rows, 1.41 MB contiguous in DRAM),
with a static per-slot budget: the host sorts each core's samples by
length, and slot j's program stores only its first BUDGETS[j] blocks --
fully-masked tail blocks are never stored, and the PJRT path's donated
zero-initialized output buffer supplies the zeros (a runtime fit check
falls back to the full-store program for inputs that exceed the
budgets).  The Pool engine runs no compute so its Q7 cores are free
for store descriptor generation.
"""

import numpy as np

import concourse.mybir as mybir
from concourse import bacc, bass
from concourse.ap import AP
from concourse.bass_utils import run_bass_kernel_spmd
from concourse.tile import TileContext

LEFT = 5
RIGHT = 5
CTXW = LEFT + RIGHT + 1  # 11
B, T, C = 32, 2000, 80
W = C * CTXW  # 880
N_CORES = 8
B_LOC = B // N_CORES  # 4 samples per core
P = 125  # SBUF partitions used per sample fold
Q = 16   # consecutive t rows per partition (P * Q == T)
QG = Q + LEFT + RIGHT  # 26 rows per partition incl. halo
TP = T + LEFT + RIGHT  # padded time length
F32 = mybir.dt.float32
I32 = mybir.dt.int32

assert P * Q == T

N_ACT = 5     # q-rows per sample computed on the scalar(ACT) engine
# store-skip granularity: NBLK partition blocks per sample; static
# per-slot store budgets. The host sorts all B samples by length (desc)
# and places rank 8j+c at core c slot j, so slot j's worst case over
# cores is the global rank-8j sample; slot j's program stores only its
# first BUDGETS[j] blocks. _fits_budget() verifies per call and falls
# back to a smaller-skip or the full-store program otherwise.
BUDGET_CFG = {
    # variant: (NBLK, per-slot budgets)
    "budget25": (25, (25, 18, 12, 6)),
    "budget": (5, (5, 4, 3, 2)),
    "full": (5, (5, 5, 5, 5)),
}


def _build_bass(variant: str = "full"):
    nc = bacc.Bacc()
    xp_dram = nc.declare_dram_parameter("xp", [B_LOC, TP, C], F32, isOutput=False)
    msk = nc.declare_dram_parameter("mask", [B_LOC, T], F32, isOutput=False)
    out = nc.declare_dram_parameter("out", [B_LOC, T, W], F32, isOutput=True)

    NBLK, budgets = BUDGET_CFG[variant]
    PBLK = P // NBLK

    with TileContext(nc) as tc:
        with (
            tc.tile_pool(name="xpool", bufs=1) as xpool,
            tc.tile_pool(name="mpool", bufs=1) as mpool,
            tc.tile_pool(name="opool", bufs=1) as opool,
        ):
            X = [None] * B_LOC
            M = [None] * B_LOC
            # prefetch all samples' inputs up front on the sync HWDGE ring
            # (separate 5-engine pool, HW descriptor generation) so they
            # never queue behind SWDGE stores
            for b in range(B_LOC):
                X[b] = xpool.tile([P, QG, C], F32, tag=f"X{b}", name=f"X{b}")
                M[b] = mpool.tile([P, Q], F32, tag=f"M{b}", name=f"M{b}")
                window = AP(
                    xp_dram[b].tensor,
                    b * TP * C,
                    [[Q * C, P], [C, QG], [1, C]],
                )
                nc.sync.dma_start(out=X[b], in_=window)
                nc.sync.dma_start(
                    out=M[b], in_=msk[b].rearrange("(p q) -> p q", q=Q)
                )

            for b in range(B_LOC):
                out_b = out[b].rearrange("(p q) w -> p q w", q=Q)
                O = opool.tile([P, Q, W], F32, tag=f"O{b % 2}", name=f"O{b}")
                for q in range(Q):
                    # dst: O[p, q, c*11+i] viewed [P, C, CTXW] (contig 880)
                    dst = O[:, q, :].rearrange("p (c i) -> p c i", i=CTXW)
                    # src: X[p, q+i, c] viewed [P, C(s1), CTXW(s80)]
                    src = X[b][:, q : q + CTXW, :].transpose([0, 2, 1])
                    mrow = M[b][:, q : q + 1]
                    if q < Q - N_ACT:
                        nc.vector.tensor_scalar_mul(out=dst, in0=src, scalar1=mrow)
                    else:
                        nc.scalar.mul(out=dst, in_=src, mul=mrow)
                for k in range(budgets[b]):
                    nc.gpsimd.dma_start(
                        out=out_b[k * PBLK : (k + 1) * PBLK],
                        in_=O[k * PBLK : (k + 1) * PBLK],
                    )
    nc.compile()
    return nc


_NC_CACHE = {}


def _get_nc(variant: str = "full"):
    if variant not in _NC_CACHE:
        _NC_CACHE[variant] = _build_bass(variant)
    return _NC_CACHE[variant]


def _balance_perm(len_abs):
    """Rank-octile slotting: sort samples by length desc; core c slot j
    gets global rank N_CORES*j + c.  Every core stores exactly
    sum(budgets) blocks, and slot j's max need is the rank-8j sample.

    Returns perm with perm[c*B_LOC + j] = original sample index."""
    order = np.argsort(-np.asarray(len_abs), kind="stable")
    perm = np.empty(B, dtype=np.int64)
    for c in range(N_CORES):
        for j in range(B_LOC):
            perm[c * B_LOC + j] = order[N_CORES * j + c]
    return perm


def _make_in_maps(x, lengths):
    x = np.asarray(x, dtype=np.float32)
    lengths = np.asarray(lengths, dtype=np.float32)
    len_abs = np.round(np.float32(T) * lengths).astype(np.int32)
    perm = _balance_perm(len_abs)
    xp_ = x[perm]
    la_ = len_abs[perm]
    x_pad = np.zeros((B, TP, C), dtype=np.float32)
    x_pad[:, LEFT : LEFT + T, :] = xp_
    mask = (np.arange(T, dtype=np.int32)[None, :] < la_[:, None]).astype(np.float32)
    in_maps = [
        {
            "xp": x_pad[c * B_LOC : (c + 1) * B_LOC],
            "mask": np.ascontiguousarray(mask[c * B_LOC : (c + 1) * B_LOC]),
        }
        for c in range(N_CORES)
    ]
    return in_maps, perm, la_


def _fits_budget(la_perm, variant):
    """la_perm: len_abs in permuted (core-major) order."""
    nblk, budgets = BUDGET_CFG[variant]
    tblk = (P // nblk) * Q
    for c in range(N_CORES):
        for j in range(B_LOC):
            if np.ceil(la_perm[c * B_LOC + j] / tblk) > budgets[j]:
                return False
    return True


def _run(x, lengths, variant: str = "auto", **spmd_kwargs):
    in_maps, perm, la_perm = _make_in_maps(x, lengths)
    if variant == "auto":
        for cand in ("budget25", "budget", "full"):
            variant = cand
            if _fits_budget(la_perm, cand):
                break
    res = run_bass_kernel_spmd(
        _get_nc(variant),
        in_maps,
        list(range(N_CORES)),
        **spmd_kwargs,
    )
    stacked = np.concatenate([r["out"] for r in res.results], axis=0)
    out = np.empty_like(stacked)
    out[perm] = stacked
    return out, res


def kernel(x, lengths):
    out, _ = _run(x, lengths)
    return out


# revision 10
# speedup vs baseline: 1.4442x; 1.4442x over previous
"""Sliding context-window gather kernel for Trainium2 (Bass/Tile).

Computes, for x[B=32, T=2000, C=80] and lengths[B]:
    out[b, t, c*11 + i] = x[b, t + i - 5, c]          (zero outside [0, T))
                          * (t < round(T * lengths[b]))
i.e. an 11-tap sliding-window gather along T with channel-major
interleave, masked by per-sample length.

Sharding: pure data-parallel, 4 samples per core across 8 cores, with a
host-side length-balanced permutation (greedy LPT on per-sample kept
rows) so every core stores a near-equal number of bytes.

Layout: host zero-pads x by 5 rows on each side of T; each sample is
loaded into SBUF as an overlapping-window view [125p x 26r x 80c] (one
DMA; partition p holds padded rows 16p..16p+25 => t = 16p-5..16p+20).

Compute: one fused op per folded q-row builds the full 880-wide
interleaved+masked output row contiguously:
    O[p, (c,i)] = X[p, q+i, c] * mask[p, q]
via tensor_scalar multiply with a per-partition scalar (the mask value)
and a transposed source access pattern (c outer stride 1, i inner
stride 80).  Rows are split between the vector and scalar(ACT) engines
(~1 us/row each, measured); aggregate compute is ~4x faster than the
store stream, so it fully hides.

DMA: this environment's HWDGE queues (sync/scalar) share only 5 SDMA
engines (~120 GB/s combined), while the gpsimd SWDGE queue spreads
across all 16 engines (~190 GB/s measured).  Loads are prefetched up
front on the sync HWDGE ring; stores go through SWDGE.  Stores are
issued per 25-partition block (400 t-rows, 1.41 MB contiguous in DRAM),
with a static per-slot budget: the host sorts each core's samples by
length, and slot j's program stores only its first BUDGETS[j] blocks --
fully-masked tail blocks are never stored, and the PJRT path's donated
zero-initialized output buffer supplies the zeros (a runtime fit check
falls back to the full-store program for inputs that exceed the
budgets).  The Pool engine runs no compute so its Q7 cores are free
for store descriptor generation.
"""

import numpy as np

import concourse.mybir as mybir
from concourse import bacc, bass
from concourse.ap import AP
from concourse.bass_utils import run_bass_kernel_spmd
from concourse.tile import TileContext

LEFT = 5
RIGHT = 5
CTXW = LEFT + RIGHT + 1  # 11
B, T, C = 32, 2000, 80
W = C * CTXW  # 880
N_CORES = 8
B_LOC = B // N_CORES  # 4 samples per core
P = 125  # SBUF partitions used per sample fold
Q = 16   # consecutive t rows per partition (P * Q == T)
QG = Q + LEFT + RIGHT  # 26 rows per partition incl. halo
TP = T + LEFT + RIGHT  # padded time length
F32 = mybir.dt.float32
BF16 = mybir.dt.bfloat16
I32 = mybir.dt.int32

assert P * Q == T

N_ACT = 5     # q-rows per sample computed on the scalar(ACT) engine
# store-skip granularity: NBLK partition blocks per sample; static
# per-slot store budgets. The host sorts all B samples by length (desc)
# and places rank 8j+c at core c slot j, so slot j's worst case over
# cores is the global rank-8j sample; slot j's program stores only its
# first BUDGETS[j] blocks. _fits_budget() verifies per call and falls
# back to a smaller-skip or the full-store program otherwise.
BUDGET_CFG = {
    # variant: (NBLK, per-slot budgets)
    "budget25": (25, (25, 18, 12, 6)),
    "budget": (5, (5, 4, 3, 2)),
    "full": (5, (5, 5, 5, 5)),
}


def _build_bass(variant: str = "full"):
    nc = bacc.Bacc()
    xp_dram = nc.declare_dram_parameter("xp", [B_LOC, TP, C], F32, isOutput=False)
    msk = nc.declare_dram_parameter("mask", [B_LOC, T], F32, isOutput=False)
    out = nc.declare_dram_parameter("out", [B_LOC, T, W], F32, isOutput=True)

    bf16o = False
    if variant.endswith("_bf16o"):
        bf16o = True
        variant = variant[: -len("_bf16o")]
    NBLK, budgets = BUDGET_CFG[variant]
    PBLK = P // NBLK
    odt = BF16 if bf16o else F32
    oring = 4 if bf16o else 2

    with TileContext(nc) as tc:
        with (
            tc.tile_pool(name="xpool", bufs=1) as xpool,
            tc.tile_pool(name="mpool", bufs=1) as mpool,
            tc.tile_pool(name="opool", bufs=1) as opool,
        ):
            X = [None] * B_LOC
            M = [None] * B_LOC
            # prefetch all samples' inputs up front on the sync HWDGE ring
            # (separate 5-engine pool, HW descriptor generation) so they
            # never queue behind SWDGE stores
            for b in range(B_LOC):
                X[b] = xpool.tile([P, QG, C], F32, tag=f"X{b}", name=f"X{b}")
                M[b] = mpool.tile([P, Q], F32, tag=f"M{b}", name=f"M{b}")
                window = AP(
                    xp_dram[b].tensor,
                    b * TP * C,
                    [[Q * C, P], [C, QG], [1, C]],
                )
                nc.sync.dma_start(out=X[b], in_=window)
                nc.sync.dma_start(
                    out=M[b], in_=msk[b].rearrange("(p q) -> p q", q=Q)
                )

            for b in range(B_LOC):
                out_b = out[b].rearrange("(p q) w -> p q w", q=Q)
                O = opool.tile([P, Q, W], odt, tag=f"O{b % oring}", name=f"O{b}")
                for q in range(Q):
                    # dst: O[p, q, c*11+i] viewed [P, C, CTXW] (contig 880)
                    dst = O[:, q, :].rearrange("p (c i) -> p c i", i=CTXW)
                    # src: X[p, q+i, c] viewed [P, C(s1), CTXW(s80)]
                    src = X[b][:, q : q + CTXW, :].transpose([0, 2, 1])
                    mrow = M[b][:, q : q + 1]
                    if q < Q - N_ACT:
                        nc.vector.tensor_scalar_mul(out=dst, in0=src, scalar1=mrow)
                    else:
                        nc.scalar.mul(out=dst, in_=src, mul=mrow)
                for k in range(budgets[b]):
                    nc.gpsimd.dma_start(
                        out=out_b[k * PBLK : (k + 1) * PBLK],
                        in_=O[k * PBLK : (k + 1) * PBLK],
                    )
    nc.compile()
    return nc


_NC_CACHE = {}


def _get_nc(variant: str = "full"):
    if variant not in _NC_CACHE:
        _NC_CACHE[variant] = _build_bass(variant)
    return _NC_CACHE[variant]


def _budget_variant_of(variant):
    return variant[: -len("_bf16o")] if variant.endswith("_bf16o") else variant


def _balance_perm(len_abs):
    """Rank-octile slotting: sort samples by length desc; core c slot j
    gets global rank N_CORES*j + c.  Every core stores exactly
    sum(budgets) blocks, and slot j's max need is the rank-8j sample.

    Returns perm with perm[c*B_LOC + j] = original sample index."""
    order = np.argsort(-np.asarray(len_abs), kind="stable")
    perm = np.empty(B, dtype=np.int64)
    for c in range(N_CORES):
        for j in range(B_LOC):
            perm[c * B_LOC + j] = order[N_CORES * j + c]
    return perm


def _make_in_maps(x, lengths):
    x = np.asarray(x, dtype=np.float32)
    lengths = np.asarray(lengths, dtype=np.float32)
    len_abs = np.round(np.float32(T) * lengths).astype(np.int32)
    perm = _balance_perm(len_abs)
    xp_ = x[perm]
    la_ = len_abs[perm]
    x_pad = np.zeros((B, TP, C), dtype=np.float32)
    x_pad[:, LEFT : LEFT + T, :] = xp_
    mask = (np.arange(T, dtype=np.int32)[None, :] < la_[:, None]).astype(np.float32)
    in_maps = [
        {
            "xp": x_pad[c * B_LOC : (c + 1) * B_LOC],
            "mask": np.ascontiguousarray(mask[c * B_LOC : (c + 1) * B_LOC]),
        }
        for c in range(N_CORES)
    ]
    return in_maps, perm, la_


def _fits_budget(la_perm, variant):
    """la_perm: len_abs in permuted (core-major) order."""
    nblk, budgets = BUDGET_CFG[_budget_variant_of(variant)]
    tblk = (P // nblk) * Q
    for c in range(N_CORES):
        for j in range(B_LOC):
            if np.ceil(la_perm[c * B_LOC + j] / tblk) > budgets[j]:
                return False
    return True


def _run(x, lengths, variant: str = "auto", **spmd_kwargs):
    in_maps, perm, la_perm = _make_in_maps(x, lengths)
    if variant == "auto":
        for cand in ("budget", "full"):
            variant = cand
            if _fits_budget(la_perm, cand):
                break
    res = run_bass_kernel_spmd(
        _get_nc(variant),
        in_maps,
        list(range(N_CORES)),
        **spmd_kwargs,
    )
    stacked = np.concatenate([r["out"] for r in res.results], axis=0)
    out = np.empty_like(stacked)
    out[perm] = stacked
    return out, res


def kernel(x, lengths):
    out, _ = _run(x, lengths)
    return out
